# revision 1
# baseline (speedup 1.0000x reference)
"""ARX RNN + exponential smoothing on 8 Trainium2 cores.

Model (per batch row):
  warmup: 32 SimpleRNN steps over inputs[:, :32, :2]
  autoregressive: 223 steps where x_t = [exog_t, pred_{t-1}],
  pred_t = h_t @ Wd + bd, output = exp-smoothed preds.

Device mapping (data-parallel, 512 batch rows per core):
  - hidden state kept transposed: hT [H=128 partitions, batch free]
  - pred feedback folded into the recurrent weight:
      Whp = Wh + Wd @ Wx[1:2,:],  bp = b + bd*Wx[1,:]
    so the per-step critical path is one K=128 matmul + tanh.
  - exog enters via a K=1 matmul (outer product) prefetched into PSUM
    one step ahead.  Matmul operands must start at partition 0/32/64,
    so exog rows cycle over those three partitions.
  - batch split in two 256-wide chunks so tanh(chunk A) overlaps
    matmuls(chunk B).
  - the PE instruction order is pinned with explicit ordering deps:
    [mmH_A(n), mmE_A(n+1), mmP_A(n-1), mmH_B(n), mmE_B(n+1), mmP_B(n-1)]
    keeping only the unavoidable tanh waits on the in-order PE stream.
  - per-step pred row (h @ Wd) lands in PSUM [1,512], DVE-copies to a
    staging tile, then a DMA (alternating gpsimd/sync queues) places it
    at partition k%128 of the preds buffer (compute engines cannot
    write arbitrary partition starts, DMA can).  The final four preds
    (k>=220) instead go DVE-direct to rows {0,32,64,96} of a third
    preds block -- legal partition starts -- skipping the ~3us
    copy+DMA+semaphore tail on the critical path.
  - exponential smoothing is a linear operator over the pred sequence:
    out = L @ preds with L lower-triangular built on host from alpha
    (columns permuted to match the preds storage layout); computed as
    3 K-chunks x 2 M-chunks of matmuls, the first K-chunk issued early
    (mid-AR, once preds 0..127 have landed) to shorten the tail.
"""

import numpy as np

import concourse.bacc as bacc
import concourse.tile as tile
from concourse import mybir
from concourse import bass_utils

F32 = mybir.dt.float32
F32R = mybir.dt.float32r
TANH = mybir.ActivationFunctionType.Tanh

B, T, LAGS, H, F = 4096, 256, 32, 128, 2
NCORES = 8
BL = B // NCORES          # 512 batch rows per core
NAR = T - LAGS - 1        # 223 autoregressive steps
NPRED = T - LAGS          # 224 pred rows / output steps

EBLK = (NAR + 2) // 3     # 75 exog column blocks
WBLK = (LAGS + 2) // 3    # 11 warmup column blocks

NDIRECT = 4               # final preds written DVE-direct (rows 0/32/64/96)
EARLY_N = 170             # AR iteration at which the first ES K-chunk runs (preds 0..127 land ~n=131; extra margin for real-HW DMA latency)

_PROGRAM_CACHE = {}
_CHAIN_MODE = True


def _build_program(nar=NAR, reps=1, chunks=2):
    key = (nar, reps, chunks)
    if key in _PROGRAM_CACHE:
        return _PROGRAM_CACHE[key]

    nc = bacc.Bacc("TRN2", target_bir_lowering=False, debug=False)

    wts = nc.dram_tensor("wts", [128, 385], F32R, kind="ExternalInput")
    lt = nc.dram_tensor("lt", [128, 672], F32R, kind="ExternalInput")
    bias2 = nc.dram_tensor("bias2", [128, 2], F32, kind="ExternalInput")
    warm = nc.dram_tensor("warm", [6, WBLK * 512], F32R, kind="ExternalInput")
    exg = nc.dram_tensor("exg", [3, EBLK * 512], F32R, kind="ExternalInput")
    out = nc.dram_tensor("es_out", [224, 512], F32, kind="ExternalOutput")

    pe_prev = [None]
    act_prev = [None]

    def chain(prev_box, inst):
        if _CHAIN_MODE and prev_box[0] is not None:
            tile.add_dep_helper(inst.ins, prev_box[0].ins, sync=False, reason="order")
        prev_box[0] = inst
        return inst

    with tile.TileContext(nc) as tc:
        with (
            tc.tile_pool(name="const", bufs=1) as const,
            tc.tile_pool(name="hA", bufs=3) as hApool,
            tc.tile_pool(name="hB", bufs=3) as hBpool,
            tc.tile_pool(name="stg", bufs=5) as stgpool,
            tc.tile_pool(name="psA", bufs=2, space="PSUM") as psA,
            tc.tile_pool(name="psB", bufs=2, space="PSUM") as psB,
            tc.tile_pool(name="psP", bufs=2, space="PSUM") as psPpool,
            tc.tile_pool(name="psE", bufs=2, space="PSUM") as psE,
        ):
            # --- loads ordered so warmup can start ASAP: wts, bias, warm
            # block 0 first; bulk of warm next; lt; exog on the gpsimd queue.
            wts_sb = const.tile([128, 385], F32R)
            nc.sync.dma_start(wts_sb, wts[:])
            bias_sb = const.tile([128, 2], F32)
            nc.sync.dma_start(bias_sb, bias2[:])
            warm_sb = const.tile([128, WBLK * 512], F32R, tag="warm")
            for g in range(3):
                nc.sync.dma_start(warm_sb[32 * g : 32 * g + 2, 0:512], warm[2 * g : 2 * g + 2, 0:512])
            for g in range(3):
                nc.sync.dma_start(warm_sb[32 * g : 32 * g + 2, 512:], warm[2 * g : 2 * g + 2, 512:])
            lt_sb = const.tile([128, 672], F32R)
            nc.sync.dma_start(lt_sb, lt[:])
            exg_sb = const.tile([128, EBLK * 512], F32R)
            for g in range(3):
                nc.gpsimd.dma_start(exg_sb[32 * g : 32 * g + 1, :], exg[g : g + 1, :])
            preds_sb = const.tile([128, 1024], F32R)
            zero_sb = const.tile([128, 512], F32, tag="z")
            nc.vector.memset(zero_sb, 0.0)

            Wh_ap = wts_sb[:, 0:128]
            Whp_ap = wts_sb[:, 128:256]
            Wd_ap = wts_sb[:, 256:257]
            b_ap = bias_sb[:, 0:1]
            bp_ap = bias_sb[:, 1:2]

            def wx_blk(p, k):
                return wts_sb[p : p + k, 257:385]

            CW = 512 // chunks
            CH = ((0, hApool, psA), (256, hBpool, psB))[:chunks]
            h_prev = [None, None]   # h_{cur} per chunk
            ps_cur = [None, None]   # PSUM tile holding pre-activation of current step
            ps_next = [None, None]  # PSUM tile with injected x for next step

            def emit_inject_warm(t):
                g, blk = t % 3, t // 3
                p = 32 * g
                for ci, (off, hp, psp) in enumerate(CH):
                    ps = psp.tile([128, CW], F32, tag="h", name=f"psw{t}_{ci}")
                    chain(pe_prev, nc.tensor.matmul(
                        ps,
                        wx_blk(p, 2),
                        warm_sb[p : p + 2, blk * 512 + off : blk * 512 + off + CW],
                        start=True,
                        stop=(t == 0),
                    ))
                    ps_next[ci] = ps

            def emit_tanh(bias_ap, name):
                for ci, (off, hp, psp) in enumerate(CH):
                    hn = hp.tile([128, CW], F32R, tag="h", name=f"{name}_{ci}")
                    chain(act_prev, nc.scalar.activation(hn, ps_cur[ci], TANH, bias=bias_ap))
                    h_prev[ci] = hn

            # ---------------- warmup: t = 0..31 ----------------
            emit_inject_warm(0)
            ps_cur, ps_next = ps_next, [None, None]
            emit_inject_warm(1)
            emit_tanh(b_ap, "hw0")
            for t in range(1, 32):
                for ci in range(len(CH)):
                    chain(pe_prev, nc.tensor.matmul(
                        ps_next[ci], Wh_ap, h_prev[ci], start=False, stop=True
                    ))
                ps_cur, ps_next = ps_next, [None, None]
                if t < 31:
                    emit_inject_warm(t + 1)
                elif nar >= 1:
                    # exog inject for AR step 1
                    for ci, (off, hp, psp) in enumerate(CH):
                        ps = psp.tile([128, CW], F32, tag="h", name=f"psa1_{ci}")
                        chain(pe_prev, nc.tensor.matmul(
                            ps, wx_blk(0, 1), exg_sb[0:1, off : off + CW],
                            start=True, stop=False,
                        ))
                        ps_next[ci] = ps
                emit_tanh(b_ap, f"hw{t}")

            # warmup inputs are dead now; reuse their slot for the third
            # preds block (final NDIRECT preds at rows 0/32/64/96).
            preds2 = const.tile([128, 512], F32R, tag="warm", name="preds2")
            nc.vector.tensor_copy(preds2, zero_sb)

            # ------------- autoregressive: n = 1..nar -------------
            # At the top of iteration n: h_prev == h_{n-1} (pred index n-1),
            # ps_next holds the exog injection for step n.
            pred_dma_ct = [0]
            direct0 = nar - (NDIRECT - 1)  # first DVE-direct pred index

            def emit_pred_tail(k, psp_t):
                if k >= direct0:
                    j = k - direct0
                    nc.vector.tensor_copy(preds2[32 * j : 32 * j + 1, :], psp_t)
                    return
                stg = stgpool.tile([1, 512], F32R, tag="s", name=f"stg{k}")
                nc.vector.tensor_copy(stg, psp_t)
                dst = preds_sb[k % 128 : k % 128 + 1, (k // 128) * 512 : (k // 128) * 512 + 512]
                eng = nc.gpsimd if pred_dma_ct[0] % 2 == 0 else nc.sync
                eng.dma_start(dst, stg)
                pred_dma_ct[0] += 1

            ES_M = ((0, 128), (128, 96))
            pse = []
            for m, (m0, mlen) in enumerate(ES_M):
                ps_m = psE.tile([128, 512], F32, tag="e", name=f"pse{m}")
                pse.append((ps_m, mlen))

            def emit_es(m, kchunk):
                ps_m, mlen = pse[m]
                m0 = ES_M[m][0]
                if kchunk == 0:
                    lhsT = lt_sb[0:128, m0 : m0 + mlen]
                    rhs = preds_sb[:, 0:512]
                    start = True
                elif kchunk == 1:
                    lhsT = lt_sb[0:92, 224 + m0 : 224 + m0 + mlen]
                    rhs = preds_sb[0:92, 512:1024]
                    start = False
                else:
                    lhsT = lt_sb[0:128, 448 + m0 : 448 + m0 + mlen]
                    rhs = preds2[:, :]
                    start = False
                # L is lower-triangular: es rows 0..127 (m=0) only involve
                # preds 0..127, so m0's k1/k2 weight columns are all zero and
                # its k0 matmul is the complete result.
                stop = (kchunk == 2) or (m == 0 and kchunk == 0 and early_es)
                chain(pe_prev, nc.tensor.matmul(
                    ps_m[0:mlen, :], lhsT, rhs,
                    start=start, stop=stop, skip_group_check=True,
                ))

            early_es = nar == NAR and reps == 1

            for rep in range(reps):
              for n in range(1, nar + 1):
                ps_cur, ps_next = ps_next, [None, None]
                psp_t = psPpool.tile([1, 512], F32, tag="p", name=f"psp{rep}_{n-1}")
                for ci, (off, hp, psp) in enumerate(CH):
                    chain(pe_prev, nc.tensor.matmul(
                        ps_cur[ci], Whp_ap, h_prev[ci], start=False, stop=True
                    ))
                    if n < nar:
                        g, blk = n % 3, n // 3
                        p = 32 * g
                        ps = psp.tile([128, CW], F32, tag="h", name=f"psa{n+1}_{ci}")
                        chain(pe_prev, nc.tensor.matmul(
                            ps,
                            wx_blk(p, 1),
                            exg_sb[p : p + 1, blk * 512 + off : blk * 512 + off + CW],
                            start=True,
                            stop=False,
                        ))
                        ps_next[ci] = ps
                    chain(pe_prev, nc.tensor.matmul(
                        psp_t[0:1, off : off + CW], Wd_ap, h_prev[ci], start=True, stop=True
                    ))
                emit_tanh(bp_ap, f"ha{rep}_{n}")
                emit_pred_tail(n - 1, psp_t)
                if early_es and EARLY_N <= n < EARLY_N + 2:
                    emit_es(n - EARLY_N, 0)
                if early_es and n == EARLY_N + 2:
                    # es rows 0..127 are final: ship the m0 output half now
                    out_m0 = const.tile([128, 512], F32, tag="z", name="out_m0")
                    nc.vector.tensor_copy(out_m0, pse[0][0])
                    nc.sync.dma_start(out[0:128, :], out_m0)
                if n == nar and rep < reps - 1:
                    for ci, (off, hp, psp) in enumerate(CH):
                        ps = psp.tile([128, CW], F32, tag="h", name=f"psr{rep}_{ci}")
                        chain(pe_prev, nc.tensor.matmul(
                            ps, wx_blk(0, 1), exg_sb[0:1, off : off + CW],
                            start=True, stop=False,
                        ))
                        ps_next[ci] = ps

            # pred for the final h
            psp_t = psPpool.tile([1, 512], F32, tag="p", name=f"psp{nar}")
            for ci, (off, _, _) in enumerate(CH):
                chain(pe_prev, nc.tensor.matmul(
                    psp_t[0:1, off : off + CW], Wd_ap, h_prev[ci], start=True, stop=True
                ))
            emit_pred_tail(nar, psp_t)

            # ------------- exponential smoothing: out = L @ preds -------------
            if not early_es:
                emit_es(0, 0)
                emit_es(1, 0)
                emit_es(0, 1)
                emit_es(0, 2)
            emit_es(1, 1)
            emit_es(1, 2)
            # preds2/warm slot is free after the k2 matmuls: stage the output
            out_stage = const.tile([128, 1024], F32, tag="warm", name="out_stage")
            ps0, len0 = pse[0]
            ps1, len1 = pse[1]
            if not early_es:
                nc.vector.tensor_copy(out_stage[0:len0, 0:512], ps0[0:len0, :])
                nc.sync.dma_start(out[0:128, :], out_stage[:, 0:512])
            nc.scalar.copy(out_stage[0:len1, 512:1024], ps1[0:len1, :])
            nc.sync.dma_start(out[128:224, :], out_stage[0:96, 512:1024])

    nc.compile()
    _PROGRAM_CACHE[key] = nc
    return nc


def _host_prep(inputs, Wx, Wh, b, Wd, bd, alpha):
    """Build the packed per-core input arrays."""
    inputs = np.ascontiguousarray(np.asarray(inputs, np.float32))
    Wx = np.asarray(Wx, np.float32)
    Wh = np.asarray(Wh, np.float32)
    b = np.asarray(b, np.float32)
    Wd = np.asarray(Wd, np.float32)
    bd_s = float(np.asarray(bd, np.float32).reshape(-1)[0])
    a = float(np.clip(np.asarray(alpha, np.float32).reshape(-1)[0], 0.0, 1.0))

    Whp = Wh + np.outer(Wd[:, 0], Wx[1, :])
    bp = b + bd_s * Wx[1, :]

    # wts: [128, 385] = Wh | Whp | Wd | Wx-block
    wts = np.zeros((128, 385), np.float32)
    wts[:, 0:128] = Wh
    wts[:, 128:256] = Whp
    wts[:, 256:257] = Wd
    for g in range(3):
        wts[32 * g : 32 * g + 2, 257:385] = Wx

    # L: smoothing operator [224, 224]
    La = np.zeros((NPRED, NPRED), np.float64)
    La[0, 0] = 1.0
    pw = (1.0 - a) ** np.arange(NPRED)
    for n in range(1, NPRED):
        La[n, 0] = pw[n]
        La[n, 1 : n + 1] = a * pw[:n][::-1]
    La = La.astype(np.float32)
    LaT = La.T  # [k, n]
    # lt: [128, 672] matching the preds storage layout:
    #   cols   0:224 -> preds k=0..127   at rows 0..127 of preds block 0
    #   cols 224:448 -> preds k=128..219 at rows 0..91  of preds block 1
    #   cols 448:672 -> preds k=220..223 at rows 0/32/64/96 of preds2
    direct0 = NAR - (NDIRECT - 1)  # 220
    lt = np.zeros((128, 672), np.float32)
    lt[:, 0:224] = LaT[0:128, :]
    lt[0 : direct0 - 128, 224:448] = LaT[128:direct0, :]
    for j in range(NDIRECT):
        lt[32 * j, 448:672] = LaT[direct0 + j, :]

    bias2 = np.zeros((128, 2), np.float32)
    bias2[:, 0] = b
    bias2[:, 1] = bp

    # per-core warm / exog packs
    # warm rows 2g+f = feature f of steps t with t%3==g, at column block t//3
    # exog rows g    = exog of steps r with r%3==g, at column block r//3
    x_w = inputs[:, :LAGS, :]              # [B, 32, 2]
    x_e = inputs[:, LAGS : LAGS + NAR, 0]  # [B, 223]
    in_maps = []
    shared = {"wts": wts, "lt": lt, "bias2": bias2}
    for c in range(NCORES):
        sw = x_w[c * BL : (c + 1) * BL]  # [512, 32, 2]
        se = x_e[c * BL : (c + 1) * BL]  # [512, 223]
        wpk = np.zeros((6, WBLK * 512), np.float32)
        for t in range(LAGS):
            g, blk = t % 3, t // 3
            wpk[2 * g, blk * 512 : (blk + 1) * 512] = sw[:, t, 0]
            wpk[2 * g + 1, blk * 512 : (blk + 1) * 512] = sw[:, t, 1]
        epk = np.zeros((3, EBLK * 512), np.float32)
        for r in range(NAR):
            g, blk = r % 3, r // 3
            epk[g, blk * 512 : (blk + 1) * 512] = se[:, r]
        in_maps.append({**shared, "warm": wpk, "exg": epk})
    return in_maps, bd_s


def kernel(inputs, Wx, Wh, b, Wd, bd, alpha, lags, _trace=False):
    assert int(lags) == LAGS
    in_maps, bd_s = _host_prep(inputs, Wx, Wh, b, Wd, bd, alpha)
    nc = _build_program()
    import time as _time

    t0 = _time.monotonic()
    try:
        res = bass_utils.run_bass_kernel_spmd(
            nc, in_maps, core_ids=list(range(NCORES)), trace=_trace
        )
    except ModuleNotFoundError:
        # NTFF profiling hook unavailable (trimmed axon client) — run plain.
        res = bass_utils.run_bass_kernel_spmd(
            nc, in_maps, core_ids=list(range(NCORES)), trace=False
        )
    kernel.last_wall_s = _time.monotonic() - t0
    es = np.stack([r["es_out"] for r in res.results])
    # es: [8, 224, 512] -> out [4096, 224, 1]
    out = es.transpose(0, 2, 1).reshape(B, NPRED, 1) + bd_s
    if _trace:
        kernel.last_results = res
    return out.astype(np.float32)



# revision 3
# speedup vs baseline: 1.0856x; 1.0856x over previous
"""ARX RNN + exponential smoothing on 8 Trainium2 cores.

Model (per batch row):
  warmup: 32 SimpleRNN steps over inputs[:, :32, :2]
  autoregressive: 223 steps where x_t = [exog_t, pred_{t-1}],
  pred_t = h_t @ Wd + bd, output = exp-smoothed preds.

Device mapping (data-parallel, 512 batch rows per core):
  - hidden state kept transposed: hT [H=128 partitions, batch free]
  - pred feedback folded into the recurrent weight:
      Whp = Wh + Wd @ Wx[1:2,:],  bp = b + bd*Wx[1,:]
    so the per-step critical path is one K=128 matmul + tanh.
  - exog enters via a K=1 matmul (outer product) prefetched into PSUM
    one step ahead.  Matmul operands must start at partition 0/32/64,
    so exog rows cycle over those three partitions.
  - batch split in two 256-wide chunks so tanh(chunk A) overlaps
    matmuls(chunk B).
  - the PE instruction order is pinned with explicit ordering deps:
    [mmH_A(n), mmE_A(n+1), mmP_A(n-1), mmH_B(n), mmE_B(n+1), mmP_B(n-1)]
    keeping only the unavoidable tanh waits on the in-order PE stream.
  - per-step pred row (h @ Wd) lands in PSUM [1,512], DVE-copies to a
    staging tile, then a DMA (alternating gpsimd/sync queues) places it
    at partition k%128 of the preds buffer (compute engines cannot
    write arbitrary partition starts, DMA can).  The final four preds
    (k>=220) instead go DVE-direct to rows {0,32,64,96} of a third
    preds block -- legal partition starts -- skipping the ~3us
    copy+DMA+semaphore tail on the critical path.
  - exponential smoothing is a linear operator over the pred sequence:
    out = L @ preds with L lower-triangular built on host from alpha
    (columns permuted to match the preds storage layout); computed as
    3 K-chunks x 2 M-chunks of matmuls, the first K-chunk issued early
    (mid-AR, once preds 0..127 have landed) to shorten the tail.
"""

import numpy as np

import concourse.bacc as bacc
import concourse.tile as tile
from concourse import mybir
from concourse import bass_utils

F32 = mybir.dt.float32
F32R = mybir.dt.float32r
TANH = mybir.ActivationFunctionType.Tanh

B, T, LAGS, H, F = 4096, 256, 32, 128, 2
NCORES = 8
BL = B // NCORES          # 512 batch rows per core
NAR = T - LAGS - 1        # 223 autoregressive steps
NPRED = T - LAGS          # 224 pred rows / output steps

EBLK = (NAR + 2) // 3     # 75 exog column blocks
WBLK = (LAGS + 2) // 3    # 11 warmup column blocks

NDIRECT = 4               # final preds written DVE-direct (rows 0/32/64/96)
EARLY_N = 170             # AR iteration at which the first ES K-chunk runs (preds 0..127 land ~n=131; extra margin for real-HW DMA latency)

_PROGRAM_CACHE = {}
_CHAIN_MODE = True


def _strip_trivial_same_engine_waits(nc):
    """Remove semaphore waits that are provably satisfied by same-engine
    in-order execution.

    Tile emits, on each engine instruction, an own-engine ordering wait
    (e.g. tanh #202 waits Activation_sem >= 201) alongside the real
    cross-engine data wait.  TRN2 allows only one wait per instruction, so
    generate_event_semaphores() splits the pair into an InstEventSemaphore
    that BLOCKS the engine's sequencer until the data sem arrives, and only
    then decodes the real instruction.  That puts ES-exec + decode (~60-80ns)
    on the tanh->matmul critical cycle every step.

    An own-engine wait whose value is already covered by the count of
    preceding same-engine updates is a no-op on in-order engines: by the
    time the instruction reaches the engine, all earlier instructions of
    that engine have executed.  Dropping those leaves one (cross-engine)
    wait on the instruction itself, so no EventSemaphore is generated and
    the instruction dispatches straight off the semaphore update.

    Only waits where the semaphore is exclusively updated by the same
    engine's (non-DMA) instruction stream are removed; DMA completions are
    asynchronous to the sequencer and are left untouched.
    """
    fn = nc.m.functions[0]
    allinsts = []
    for b in fn.blocks:
        allinsts.extend(list(b.instructions))

    upd_engines = {}
    dma_sems = set()
    for i in allinsts:
        si = i.sync_info
        if si is None:
            continue
        for u in si.on_update:
            upd_engines.setdefault(u.id, set()).add(i.engine)
            if isinstance(i, mybir.InstDMACopy):
                dma_sems.add(u.id)

    cnt = {}
    nstripped = 0
    for i in allinsts:
        si = i.sync_info
        if si is None:
            continue
        if si.on_wait and not isinstance(i, mybir.InstDMACopy):
            kept = []
            for w in si.on_wait:
                if (
                    w.sync_type == "semaphore"
                    and w.wait_mode == "sem-ge-imm"
                    and w.wait_reg is None
                    and w.id not in dma_sems
                    and upd_engines.get(w.id) == {i.engine}
                    and cnt.get((w.id, i.engine), 0) >= w.wait_value
                ):
                    nstripped += 1
                    continue
                kept.append(w)
            if len(kept) != len(si.on_wait):
                si.on_wait = kept
        for u in si.on_update:
            k = (u.id, i.engine)
            cnt[k] = cnt.get(k, 0) + u.update_value
    return nstripped


def _build_program(nar=NAR, reps=1, chunks=2):
    key = (nar, reps, chunks)
    if key in _PROGRAM_CACHE:
        return _PROGRAM_CACHE[key]

    nc = bacc.Bacc("TRN2", target_bir_lowering=False, debug=False)

    wts = nc.dram_tensor("wts", [128, 385], F32R, kind="ExternalInput")
    lt = nc.dram_tensor("lt", [128, 672], F32R, kind="ExternalInput")
    bias2 = nc.dram_tensor("bias2", [128, 2], F32, kind="ExternalInput")
    warm = nc.dram_tensor("warm", [6, WBLK * 512], F32R, kind="ExternalInput")
    exg = nc.dram_tensor("exg", [3, EBLK * 512], F32R, kind="ExternalInput")
    out = nc.dram_tensor("es_out", [224, 512], F32, kind="ExternalOutput")

    pe_prev = [None]
    act_prev = [None]

    def chain(prev_box, inst):
        if _CHAIN_MODE and prev_box[0] is not None:
            tile.add_dep_helper(inst.ins, prev_box[0].ins, sync=False, reason="order")
        prev_box[0] = inst
        return inst

    with tile.TileContext(nc) as tc:
        with (
            tc.tile_pool(name="const", bufs=1) as const,
            tc.tile_pool(name="hA", bufs=3) as hApool,
            tc.tile_pool(name="hB", bufs=3) as hBpool,
            tc.tile_pool(name="stg", bufs=5) as stgpool,
            tc.tile_pool(name="psA", bufs=2, space="PSUM") as psA,
            tc.tile_pool(name="psB", bufs=2, space="PSUM") as psB,
            tc.tile_pool(name="psP", bufs=2, space="PSUM") as psPpool,
            tc.tile_pool(name="psE", bufs=2, space="PSUM") as psE,
        ):
            # --- loads ordered so warmup can start ASAP: wts, bias, warm
            # block 0 first; bulk of warm next; lt; exog on the gpsimd queue.
            wts_sb = const.tile([128, 385], F32R)
            nc.sync.dma_start(wts_sb, wts[:])
            bias_sb = const.tile([128, 2], F32)
            nc.sync.dma_start(bias_sb, bias2[:])
            warm_sb = const.tile([128, WBLK * 512], F32R, tag="warm")
            for g in range(3):
                nc.sync.dma_start(warm_sb[32 * g : 32 * g + 2, 0:512], warm[2 * g : 2 * g + 2, 0:512])
            for g in range(3):
                nc.sync.dma_start(warm_sb[32 * g : 32 * g + 2, 512:], warm[2 * g : 2 * g + 2, 512:])
            lt_sb = const.tile([128, 672], F32R)
            nc.sync.dma_start(lt_sb, lt[:])
            exg_sb = const.tile([128, EBLK * 512], F32R)
            for g in range(3):
                nc.gpsimd.dma_start(exg_sb[32 * g : 32 * g + 1, :], exg[g : g + 1, :])
            preds_sb = const.tile([128, 1024], F32R)
            zero_sb = const.tile([128, 512], F32, tag="z")
            nc.vector.memset(zero_sb, 0.0)

            Wh_ap = wts_sb[:, 0:128]
            Whp_ap = wts_sb[:, 128:256]
            Wd_ap = wts_sb[:, 256:257]
            b_ap = bias_sb[:, 0:1]
            bp_ap = bias_sb[:, 1:2]

            def wx_blk(p, k):
                return wts_sb[p : p + k, 257:385]

            CW = 512 // chunks
            CH = ((0, hApool, psA), (256, hBpool, psB))[:chunks]
            h_prev = [None, None]   # h_{cur} per chunk
            ps_cur = [None, None]   # PSUM tile holding pre-activation of current step
            ps_next = [None, None]  # PSUM tile with injected x for next step

            def emit_inject_warm(t):
                g, blk = t % 3, t // 3
                p = 32 * g
                for ci, (off, hp, psp) in enumerate(CH):
                    ps = psp.tile([128, CW], F32, tag="h", name=f"psw{t}_{ci}")
                    chain(pe_prev, nc.tensor.matmul(
                        ps,
                        wx_blk(p, 2),
                        warm_sb[p : p + 2, blk * 512 + off : blk * 512 + off + CW],
                        start=True,
                        stop=(t == 0),
                    ))
                    ps_next[ci] = ps

            def emit_tanh(bias_ap, name):
                for ci, (off, hp, psp) in enumerate(CH):
                    hn = hp.tile([128, CW], F32R, tag="h", name=f"{name}_{ci}")
                    chain(act_prev, nc.scalar.activation(hn, ps_cur[ci], TANH, bias=bias_ap))
                    h_prev[ci] = hn

            # ---------------- warmup: t = 0..31 ----------------
            emit_inject_warm(0)
            ps_cur, ps_next = ps_next, [None, None]
            emit_inject_warm(1)
            emit_tanh(b_ap, "hw0")
            for t in range(1, 32):
                for ci in range(len(CH)):
                    chain(pe_prev, nc.tensor.matmul(
                        ps_next[ci], Wh_ap, h_prev[ci], start=False, stop=True
                    ))
                ps_cur, ps_next = ps_next, [None, None]
                if t < 31:
                    emit_inject_warm(t + 1)
                elif nar >= 1:
                    # exog inject for AR step 1
                    for ci, (off, hp, psp) in enumerate(CH):
                        ps = psp.tile([128, CW], F32, tag="h", name=f"psa1_{ci}")
                        chain(pe_prev, nc.tensor.matmul(
                            ps, wx_blk(0, 1), exg_sb[0:1, off : off + CW],
                            start=True, stop=False,
                        ))
                        ps_next[ci] = ps
                emit_tanh(b_ap, f"hw{t}")

            # warmup inputs are dead now; reuse their slot for the third
            # preds block (final NDIRECT preds at rows 0/32/64/96).
            preds2 = const.tile([128, 512], F32R, tag="warm", name="preds2")
            nc.vector.tensor_copy(preds2, zero_sb)

            # ------------- autoregressive: n = 1..nar -------------
            # At the top of iteration n: h_prev == h_{n-1} (pred index n-1),
            # ps_next holds the exog injection for step n.
            pred_dma_ct = [0]
            direct0 = nar - (NDIRECT - 1)  # first DVE-direct pred index

            def emit_pred_tail(k, psp_t):
                if k >= direct0:
                    j = k - direct0
                    nc.vector.tensor_copy(preds2[32 * j : 32 * j + 1, :], psp_t)
                    return
                stg = stgpool.tile([1, 512], F32R, tag="s", name=f"stg{k}")
                nc.vector.tensor_copy(stg, psp_t)
                dst = preds_sb[k % 128 : k % 128 + 1, (k // 128) * 512 : (k // 128) * 512 + 512]
                eng = nc.gpsimd if pred_dma_ct[0] % 2 == 0 else nc.sync
                eng.dma_start(dst, stg)
                pred_dma_ct[0] += 1

            ES_M = ((0, 128), (128, 96))
            pse = []
            for m, (m0, mlen) in enumerate(ES_M):
                ps_m = psE.tile([128, 512], F32, tag="e", name=f"pse{m}")
                pse.append((ps_m, mlen))

            def emit_es(m, kchunk):
                ps_m, mlen = pse[m]
                m0 = ES_M[m][0]
                if kchunk == 0:
                    lhsT = lt_sb[0:128, m0 : m0 + mlen]
                    rhs = preds_sb[:, 0:512]
                    start = True
                elif kchunk == 1:
                    lhsT = lt_sb[0:92, 224 + m0 : 224 + m0 + mlen]
                    rhs = preds_sb[0:92, 512:1024]
                    start = False
                else:
                    lhsT = lt_sb[0:128, 448 + m0 : 448 + m0 + mlen]
                    rhs = preds2[:, :]
                    start = False
                # L is lower-triangular: es rows 0..127 (m=0) only involve
                # preds 0..127, so m0's k1/k2 weight columns are all zero and
                # its k0 matmul is the complete result.
                stop = (kchunk == 2) or (m == 0 and kchunk == 0 and early_es)
                chain(pe_prev, nc.tensor.matmul(
                    ps_m[0:mlen, :], lhsT, rhs,
                    start=start, stop=stop, skip_group_check=True,
                ))

            early_es = nar == NAR and reps == 1

            for rep in range(reps):
              for n in range(1, nar + 1):
                ps_cur, ps_next = ps_next, [None, None]
                psp_t = psPpool.tile([1, 512], F32, tag="p", name=f"psp{rep}_{n-1}")
                for ci, (off, hp, psp) in enumerate(CH):
                    chain(pe_prev, nc.tensor.matmul(
                        ps_cur[ci], Whp_ap, h_prev[ci], start=False, stop=True
                    ))
                    if n < nar:
                        g, blk = n % 3, n // 3
                        p = 32 * g
                        ps = psp.tile([128, CW], F32, tag="h", name=f"psa{n+1}_{ci}")
                        chain(pe_prev, nc.tensor.matmul(
                            ps,
                            wx_blk(p, 1),
                            exg_sb[p : p + 1, blk * 512 + off : blk * 512 + off + CW],
                            start=True,
                            stop=False,
                        ))
                        ps_next[ci] = ps
                    chain(pe_prev, nc.tensor.matmul(
                        psp_t[0:1, off : off + CW], Wd_ap, h_prev[ci], start=True, stop=True
                    ))
                emit_tanh(bp_ap, f"ha{rep}_{n}")
                emit_pred_tail(n - 1, psp_t)
                if early_es and EARLY_N <= n < EARLY_N + 2:
                    emit_es(n - EARLY_N, 0)
                if early_es and n == EARLY_N + 2:
                    # es rows 0..127 are final: ship the m0 output half now
                    out_m0 = const.tile([128, 512], F32, tag="z", name="out_m0")
                    nc.vector.tensor_copy(out_m0, pse[0][0])
                    nc.sync.dma_start(out[0:128, :], out_m0)
                if n == nar and rep < reps - 1:
                    for ci, (off, hp, psp) in enumerate(CH):
                        ps = psp.tile([128, CW], F32, tag="h", name=f"psr{rep}_{ci}")
                        chain(pe_prev, nc.tensor.matmul(
                            ps, wx_blk(0, 1), exg_sb[0:1, off : off + CW],
                            start=True, stop=False,
                        ))
                        ps_next[ci] = ps

            # pred for the final h
            psp_t = psPpool.tile([1, 512], F32, tag="p", name=f"psp{nar}")
            for ci, (off, _, _) in enumerate(CH):
                chain(pe_prev, nc.tensor.matmul(
                    psp_t[0:1, off : off + CW], Wd_ap, h_prev[ci], start=True, stop=True
                ))
            emit_pred_tail(nar, psp_t)

            # ------------- exponential smoothing: out = L @ preds -------------
            if not early_es:
                emit_es(0, 0)
                emit_es(1, 0)
                emit_es(0, 1)
                emit_es(0, 2)
            emit_es(1, 1)
            emit_es(1, 2)
            # preds2/warm slot is free after the k2 matmuls: stage the output
            out_stage = const.tile([128, 1024], F32, tag="warm", name="out_stage")
            ps0, len0 = pse[0]
            ps1, len1 = pse[1]
            if not early_es:
                nc.vector.tensor_copy(out_stage[0:len0, 0:512], ps0[0:len0, :])
                nc.sync.dma_start(out[0:128, :], out_stage[:, 0:512])
            nc.scalar.copy(out_stage[0:len1, 512:1024], ps1[0:len1, :])
            nc.sync.dma_start(out[128:224, :], out_stage[0:96, 512:1024])

    _strip_trivial_same_engine_waits(nc)
    nc.compile()
    _PROGRAM_CACHE[key] = nc
    return nc


def _host_prep(inputs, Wx, Wh, b, Wd, bd, alpha):
    """Build the packed per-core input arrays."""
    inputs = np.ascontiguousarray(np.asarray(inputs, np.float32))
    Wx = np.asarray(Wx, np.float32)
    Wh = np.asarray(Wh, np.float32)
    b = np.asarray(b, np.float32)
    Wd = np.asarray(Wd, np.float32)
    bd_s = float(np.asarray(bd, np.float32).reshape(-1)[0])
    a = float(np.clip(np.asarray(alpha, np.float32).reshape(-1)[0], 0.0, 1.0))

    Whp = Wh + np.outer(Wd[:, 0], Wx[1, :])
    bp = b + bd_s * Wx[1, :]

    # wts: [128, 385] = Wh | Whp | Wd | Wx-block
    wts = np.zeros((128, 385), np.float32)
    wts[:, 0:128] = Wh
    wts[:, 128:256] = Whp
    wts[:, 256:257] = Wd
    for g in range(3):
        wts[32 * g : 32 * g + 2, 257:385] = Wx

    # L: smoothing operator [224, 224]
    La = np.zeros((NPRED, NPRED), np.float64)
    La[0, 0] = 1.0
    pw = (1.0 - a) ** np.arange(NPRED)
    for n in range(1, NPRED):
        La[n, 0] = pw[n]
        La[n, 1 : n + 1] = a * pw[:n][::-1]
    La = La.astype(np.float32)
    LaT = La.T  # [k, n]
    # lt: [128, 672] matching the preds storage layout:
    #   cols   0:224 -> preds k=0..127   at rows 0..127 of preds block 0
    #   cols 224:448 -> preds k=128..219 at rows 0..91  of preds block 1
    #   cols 448:672 -> preds k=220..223 at rows 0/32/64/96 of preds2
    direct0 = NAR - (NDIRECT - 1)  # 220
    lt = np.zeros((128, 672), np.float32)
    lt[:, 0:224] = LaT[0:128, :]
    lt[0 : direct0 - 128, 224:448] = LaT[128:direct0, :]
    for j in range(NDIRECT):
        lt[32 * j, 448:672] = LaT[direct0 + j, :]

    bias2 = np.zeros((128, 2), np.float32)
    bias2[:, 0] = b
    bias2[:, 1] = bp

    # per-core warm / exog packs
    # warm rows 2g+f = feature f of steps t with t%3==g, at column block t//3
    # exog rows g    = exog of steps r with r%3==g, at column block r//3
    x_w = inputs[:, :LAGS, :]              # [B, 32, 2]
    x_e = inputs[:, LAGS : LAGS + NAR, 0]  # [B, 223]
    in_maps = []
    shared = {"wts": wts, "lt": lt, "bias2": bias2}
    for c in range(NCORES):
        sw = x_w[c * BL : (c + 1) * BL]  # [512, 32, 2]
        se = x_e[c * BL : (c + 1) * BL]  # [512, 223]
        wpk = np.zeros((6, WBLK * 512), np.float32)
        for t in range(LAGS):
            g, blk = t % 3, t // 3
            wpk[2 * g, blk * 512 : (blk + 1) * 512] = sw[:, t, 0]
            wpk[2 * g + 1, blk * 512 : (blk + 1) * 512] = sw[:, t, 1]
        epk = np.zeros((3, EBLK * 512), np.float32)
        for r in range(NAR):
            g, blk = r % 3, r // 3
            epk[g, blk * 512 : (blk + 1) * 512] = se[:, r]
        in_maps.append({**shared, "warm": wpk, "exg": epk})
    return in_maps, bd_s


def kernel(inputs, Wx, Wh, b, Wd, bd, alpha, lags, _trace=False):
    assert int(lags) == LAGS
    in_maps, bd_s = _host_prep(inputs, Wx, Wh, b, Wd, bd, alpha)
    nc = _build_program()
    import time as _time

    t0 = _time.monotonic()
    try:
        res = bass_utils.run_bass_kernel_spmd(
            nc, in_maps, core_ids=list(range(NCORES)), trace=_trace
        )
    except ModuleNotFoundError:
        # NTFF profiling hook unavailable (trimmed axon client) — run plain.
        res = bass_utils.run_bass_kernel_spmd(
            nc, in_maps, core_ids=list(range(NCORES)), trace=False
        )
    kernel.last_wall_s = _time.monotonic() - t0
    es = np.stack([r["es_out"] for r in res.results])
    # es: [8, 224, 512] -> out [4096, 224, 1]
    out = es.transpose(0, 2, 1).reshape(B, NPRED, 1) + bd_s
    if _trace:
        kernel.last_results = res
    return out.astype(np.float32)



# revision 24
# speedup vs baseline: 1.0931x; 1.0069x over previous
"""ARX RNN + exponential smoothing on 8 Trainium2 cores.

Model (per batch row):
  warmup: 32 SimpleRNN steps over inputs[:, :32, :2]
  autoregressive: 223 steps where x_t = [exog_t, pred_{t-1}],
  pred_t = h_t @ Wd + bd, output = exp-smoothed preds.

Device mapping (data-parallel, 512 batch rows per core):
  - hidden state kept transposed: hT [H=128 partitions, batch free]
  - pred feedback folded into the recurrent weight:
      Whp = Wh + Wd @ Wx[1:2,:],  bp = b + bd*Wx[1,:]
    so the per-step critical path is one K=128 matmul + tanh.
  - exog enters via a K=1 matmul (outer product) prefetched into PSUM
    one step ahead.  Matmul operands must start at partition 0/32/64,
    so exog rows cycle over those three partitions.
  - batch split in two 256-wide chunks so tanh(chunk A) overlaps
    matmuls(chunk B).
  - the PE instruction order is pinned with explicit ordering deps:
    [mmH_A(n), mmE_A(n+1), mmP_A(n-1), mmH_B(n), mmE_B(n+1), mmP_B(n-1)]
    keeping only the unavoidable tanh waits on the in-order PE stream.
  - per-step pred row (h @ Wd) lands in PSUM [1,512], DVE-copies to a
    staging tile, then a DMA (alternating gpsimd/sync queues) places it
    at partition k%128 of the preds buffer (compute engines cannot
    write arbitrary partition starts, DMA can).  The final four preds
    (k>=220) instead go DVE-direct to rows {0,32,64,96} of a third
    preds block -- legal partition starts -- skipping the ~3us
    copy+DMA+semaphore tail on the critical path.
  - exponential smoothing is a linear operator over the pred sequence:
    out = L @ preds with L lower-triangular built on host from alpha
    (columns permuted to match the preds storage layout); computed as
    3 K-chunks x 2 M-chunks of matmuls, the first K-chunk issued early
    (mid-AR, once preds 0..127 have landed) to shorten the tail.
"""

import numpy as np

import concourse.bacc as bacc
import concourse.tile as tile
from concourse import mybir
from concourse import bass_utils

F32 = mybir.dt.float32
F32R = mybir.dt.float32r
TANH = mybir.ActivationFunctionType.Tanh

B, T, LAGS, H, F = 4096, 256, 32, 128, 2
NCORES = 8
BL = B // NCORES          # 512 batch rows per core
NAR = T - LAGS - 1        # 223 autoregressive steps
NPRED = T - LAGS          # 224 pred rows / output steps

EBLK = (NAR + 2) // 3     # 75 exog column blocks
WBLK = (LAGS + 2) // 3    # 11 warmup column blocks

NDIRECT = 8               # final preds written DVE-direct (rows 0/32/64/96 of two blocks)
EARLY_N = 170             # AR iteration at which the first ES K-chunk runs (preds 0..127 land ~n=131; extra margin for real-HW DMA latency)
K1_N = 221                # AR iteration for the second ES K-chunk (preds 128..215 land ~n=219)

_PROGRAM_CACHE = {}
_CHAIN_MODE = True


def _strip_trivial_same_engine_waits(nc):
    """Remove semaphore waits that are provably satisfied by same-engine
    in-order execution.

    Tile emits, on each engine instruction, an own-engine ordering wait
    (e.g. tanh #202 waits Activation_sem >= 201) alongside the real
    cross-engine data wait.  TRN2 allows only one wait per instruction, so
    generate_event_semaphores() splits the pair into an InstEventSemaphore
    that BLOCKS the engine's sequencer until the data sem arrives, and only
    then decodes the real instruction.  That puts ES-exec + decode (~60-80ns)
    on the tanh->matmul critical cycle every step.

    An own-engine wait whose value is already covered by the count of
    preceding same-engine updates is a no-op on in-order engines: by the
    time the instruction reaches the engine, all earlier instructions of
    that engine have executed.  Dropping those leaves one (cross-engine)
    wait on the instruction itself, so no EventSemaphore is generated and
    the instruction dispatches straight off the semaphore update.

    Only waits where the semaphore is exclusively updated by the same
    engine's (non-DMA) instruction stream are removed; DMA completions are
    asynchronous to the sequencer and are left untouched.
    """
    fn = nc.m.functions[0]
    allinsts = []
    for b in fn.blocks:
        allinsts.extend(list(b.instructions))

    upd_engines = {}
    dma_sems = set()
    for i in allinsts:
        si = i.sync_info
        if si is None:
            continue
        for u in si.on_update:
            upd_engines.setdefault(u.id, set()).add(i.engine)
            if isinstance(i, mybir.InstDMACopy):
                dma_sems.add(u.id)

    cnt = {}
    nstripped = 0
    for i in allinsts:
        si = i.sync_info
        if si is None:
            continue
        if si.on_wait and not isinstance(i, mybir.InstDMACopy):
            kept = []
            for w in si.on_wait:
                if (
                    w.sync_type == "semaphore"
                    and w.wait_mode == "sem-ge-imm"
                    and w.wait_reg is None
                    and w.id not in dma_sems
                    and upd_engines.get(w.id) == {i.engine}
                    and cnt.get((w.id, i.engine), 0) >= w.wait_value
                ):
                    nstripped += 1
                    continue
                kept.append(w)
            if len(kept) != len(si.on_wait):
                si.on_wait = kept
        for u in si.on_update:
            k = (u.id, i.engine)
            cnt[k] = cnt.get(k, 0) + u.update_value
    return nstripped


def _build_program(nar=NAR, reps=1, chunks=2):
    key = (nar, reps, chunks)
    if key in _PROGRAM_CACHE:
        return _PROGRAM_CACHE[key]

    nc = bacc.Bacc("TRN2", target_bir_lowering=False, debug=False)

    # wts cols: 0:128 Wx-block | 128:640 warm block 0 | 640:768 Wh
    #           | 768:896 Whp | 896:897 Wd
    wts = nc.dram_tensor("wts", [128, 897], F32R, kind="ExternalInput")
    lt = nc.dram_tensor("lt", [128, 896], F32R, kind="ExternalInput")
    bias2 = nc.dram_tensor("bias2", [128, 2], F32, kind="ExternalInput")
    warm = nc.dram_tensor("warm", [6, (WBLK - 1) * 512], F32R, kind="ExternalInput")
    exg = nc.dram_tensor("exg", [3, EBLK * 512], F32R, kind="ExternalInput")
    out = nc.dram_tensor("es_out", [224, 512], F32, kind="ExternalOutput")

    pe_prev = [None]
    act_prev = [None]

    def chain(prev_box, inst):
        if _CHAIN_MODE and prev_box[0] is not None:
            tile.add_dep_helper(inst.ins, prev_box[0].ins, sync=False, reason="order")
        prev_box[0] = inst
        return inst

    with tile.TileContext(nc) as tc:
        with (
            tc.tile_pool(name="const", bufs=1) as const,
            tc.tile_pool(name="hA", bufs=3) as hApool,
            tc.tile_pool(name="hB", bufs=3) as hBpool,
            tc.tile_pool(name="stg", bufs=5) as stgpool,
            tc.tile_pool(name="psA", bufs=2, space="PSUM") as psA,
            tc.tile_pool(name="psB", bufs=2, space="PSUM") as psB,
            tc.tile_pool(name="psP", bufs=2, space="PSUM") as psPpool,
            tc.tile_pool(name="psE", bufs=2, space="PSUM") as psE,
        ):
            # --- critical loads fanned out over three DMA queues so the
            # warmup can start ~2.6us in: [wx|warm0] on sync, bias on the
            # scalar queue, [Wh|Whp|Wd] on the vector queue.  A dummy
            # activation pulls the 1.3us act-table load off the first
            # tanh's critical path.
            wts_sb = const.tile([128, 897], F32R)
            # only partition rows {32g, 32g+1} of the wx/warm0 region are
            # ever read; three tiny row-sliced DMAs beat one 128-row load
            # (DMA time scales with descriptor bytes).
            for g in range(3):
                nc.sync.dma_start(
                    wts_sb[32 * g : 32 * g + 2, 0:640], wts[32 * g : 32 * g + 2, 0:640]
                )
            bias_sb = const.tile([128, 2], F32)
            nc.scalar.dma_start(bias_sb, bias2[:])
            nc.gpsimd.dma_start(wts_sb[:, 640:897], wts[:, 640:897])
            z1 = const.tile([128, 1], F32)
            nc.vector.memset(z1, 0.0)
            dummy_out = const.tile([128, 1], F32R)
            nc.scalar.activation(dummy_out, z1, TANH)
            warm_sb = const.tile([128, (WBLK - 1) * 512], F32R, tag="warm")
            for g in range(3):
                nc.sync.dma_start(warm_sb[32 * g : 32 * g + 2, :], warm[2 * g : 2 * g + 2, :])
            lt_sb = const.tile([128, 896], F32R)
            nc.sync.dma_start(lt_sb, lt[:])
            exg_sb = const.tile([128, EBLK * 512], F32R)
            for g in range(3):
                nc.gpsimd.dma_start(exg_sb[32 * g : 32 * g + 1, :], exg[g : g + 1, :])
            preds_sb = const.tile([128, 1024], F32R)
            out_stage = const.tile([128, 1024], F32, name="out_stage")
            e2a_stage = const.tile([4, 256], F32, name="e2a_stage")
            e2b_stage = const.tile([4, 256], F32, name="e2b_stage")

            Wh_ap = wts_sb[:, 640:768]
            Whp_ap = wts_sb[:, 768:896]
            Wd_ap = wts_sb[:, 896:897]
            b_ap = bias_sb[:, 0:1]
            bp_ap = bias_sb[:, 1:2]

            def wx_blk(p, k):
                return wts_sb[p : p + k, 0:128]

            CW = 512 // chunks
            CH = ((0, hApool, psA), (256, hBpool, psB))[:chunks]
            h_prev = [None, None]   # h_{cur} per chunk
            ps_cur = [None, None]   # PSUM tile holding pre-activation of current step
            ps_next = [None, None]  # PSUM tile with injected x for next step

            def emit_inject_warm(t):
                g, blk = t % 3, t // 3
                p = 32 * g
                if blk == 0:
                    def wsrc(off):
                        return wts_sb[p : p + 2, 128 + off : 128 + off + CW]
                else:
                    def wsrc(off):
                        base = (blk - 1) * 512
                        return warm_sb[p : p + 2, base + off : base + off + CW]
                for ci, (off, hp, psp) in enumerate(CH):
                    ps = psp.tile([128, CW], F32, tag="h", name=f"psw{t}_{ci}")
                    chain(pe_prev, nc.tensor.matmul(
                        ps,
                        wx_blk(p, 2),
                        wsrc(off),
                        start=True,
                        stop=(t == 0),
                    ))
                    ps_next[ci] = ps

            def emit_tanh(bias_ap, name):
                for ci, (off, hp, psp) in enumerate(CH):
                    hn = hp.tile([128, CW], F32R, tag="h", name=f"{name}_{ci}")
                    chain(act_prev, nc.scalar.activation(hn, ps_cur[ci], TANH, bias=bias_ap))
                    h_prev[ci] = hn

            # ---------------- warmup: t = 0..31 ----------------
            emit_inject_warm(0)
            ps_cur, ps_next = ps_next, [None, None]
            emit_inject_warm(1)
            emit_tanh(b_ap, "hw0")
            for t in range(1, 32):
                for ci in range(len(CH)):
                    chain(pe_prev, nc.tensor.matmul(
                        ps_next[ci], Wh_ap, h_prev[ci], start=False, stop=True
                    ))
                ps_cur, ps_next = ps_next, [None, None]
                if t < 31:
                    emit_inject_warm(t + 1)
                elif nar >= 1:
                    # exog inject for AR step 1
                    for ci, (off, hp, psp) in enumerate(CH):
                        ps = psp.tile([128, CW], F32, tag="h", name=f"psa1_{ci}")
                        chain(pe_prev, nc.tensor.matmul(
                            ps, wx_blk(0, 1), exg_sb[0:1, off : off + CW],
                            start=True, stop=False,
                        ))
                        ps_next[ci] = ps
                emit_tanh(b_ap, f"hw{t}")

            # warmup inputs are dead now; reuse the warm slot for the two
            # DVE-direct preds blocks (final NDIRECT preds at partition rows
            # 0/32/64/96: cols 0:512 hold preds 216..219, cols 512:1024 hold
            # preds 220..223).
            preds23 = const.tile([128, 1024], F32R, tag="warm", name="preds23")
            nc.vector.memset(preds23[:, :].bitcast(F32), 0.0)

            # ------------- autoregressive: n = 1..nar -------------
            # At the top of iteration n: h_prev == h_{n-1} (pred index n-1),
            # ps_next holds the exog injection for step n.
            pred_dma_ct = [0]
            direct0 = nar - (NDIRECT - 1)  # first DVE-direct pred index

            def emit_pred_tail(k, psp_t):
                if k >= direct0:
                    j = k - direct0
                    half = 0 if j < 4 else 512
                    r = 32 * (j % 4)
                    nc.vector.tensor_copy(preds23[r : r + 1, half : half + 512], psp_t)
                    return
                stg = stgpool.tile([1, 512], F32R, tag="s", name=f"stg{k}")
                nc.vector.tensor_copy(stg, psp_t)
                dst = preds_sb[k % 128 : k % 128 + 1, (k // 128) * 512 : (k // 128) * 512 + 512]
                eng = nc.gpsimd if pred_dma_ct[0] % 2 == 0 else nc.sync
                eng.dma_start(dst, stg)
                pred_dma_ct[0] += 1

            # ES accumulators: pse0 = out rows 0:128 (complete after k0,
            # staged out by n=EARLY_N+3); pse1 = rows 128:220 at partitions
            # 0:92 (k0+k1+k2a).  Rows 220:224 (pse2) accumulate in
            # partitions 0:4 of pse0's bank once m0 has been staged out --
            # matmul outputs must start at partition 0/32/64 and may not
            # span quadrants, and there is no ninth PSUM bank to spare.
            pse0 = psE.tile([128, 512], F32, tag="e", name="pse0")
            pse1b = psE.tile([128, 512], F32, tag="e", name="pse1")
            pse1 = pse1b[0:92, :]
            pse2 = pse0[0:4, :]

            def emit_es(which):
                # K-chunk layout in lt cols: k0 0:224 | k1 224:448
                # | k2a 448:672 | k2b 672:896; within each region col j is
                # output row j (m0 rows 0:128, m1 rows 128:220, e2 220:224).
                k0_rhs = preds_sb[:, 0:512]
                k1_rhs = preds_sb[0:88, 512:1024]
                k2a_rhs = preds23[:, 0:512]
                k2b_rhs = preds23[:, 512:1024]
                mm = {
                    "m0k0": (pse0[0:128, :], lt_sb[0:128, 0:128], k0_rhs, True, True),
                    "m1k0": (pse1, lt_sb[0:128, 128:220], k0_rhs, True, False),
                    "e2k0": (pse2, lt_sb[0:128, 220:224], k0_rhs, True, False),
                    "m1k1": (pse1, lt_sb[0:88, 352:444], k1_rhs, False, False),
                    "e2k1": (pse2, lt_sb[0:88, 444:448], k1_rhs, False, False),
                    "m1k2a": (pse1, lt_sb[0:128, 576:668], k2a_rhs, False, True),
                    "e2k2a": (pse2, lt_sb[0:128, 668:672], k2a_rhs, False, False),
                    "e2k2b": (pse2, lt_sb[0:128, 892:896], k2b_rhs, False, True),
                }
                dst, lhsT, rhs, start, stop = mm[which]
                chain(pe_prev, nc.tensor.matmul(
                    dst, lhsT, rhs, start=start, stop=stop, skip_group_check=True,
                ))

            early_es = nar == NAR and reps == 1

            for rep in range(reps):
              for n in range(1, nar + 1):
                ps_cur, ps_next = ps_next, [None, None]
                psp_t = psPpool.tile([1, 512], F32, tag="p", name=f"psp{rep}_{n-1}")
                for ci, (off, hp, psp) in enumerate(CH):
                    chain(pe_prev, nc.tensor.matmul(
                        ps_cur[ci], Whp_ap, h_prev[ci], start=False, stop=True
                    ))
                    if n < nar:
                        g, blk = n % 3, n // 3
                        p = 32 * g
                        ps = psp.tile([128, CW], F32, tag="h", name=f"psa{n+1}_{ci}")
                        chain(pe_prev, nc.tensor.matmul(
                            ps,
                            wx_blk(p, 1),
                            exg_sb[p : p + 1, blk * 512 + off : blk * 512 + off + CW],
                            start=True,
                            stop=False,
                        ))
                        ps_next[ci] = ps
                    chain(pe_prev, nc.tensor.matmul(
                        psp_t[0:1, off : off + CW], Wd_ap, h_prev[ci], start=True, stop=True
                    ))
                emit_tanh(bp_ap, f"ha{rep}_{n}")
                emit_pred_tail(n - 1, psp_t)
                if early_es:
                    # one ES matmul per step: two at once overruns the
                    # PE's ~210ns/step slack and stretches the period.
                    if n == EARLY_N:
                        emit_es("m0k0")
                    elif n == EARLY_N + 1:
                        emit_es("m1k0")
                    elif n == EARLY_N + 2:
                        # es rows 0..127 final: stage and ship
                        nc.vector.tensor_copy(out_stage[:, 0:512], pse0[0:128, :])
                        nc.sync.dma_start(out[0:128, :], out_stage[:, 0:512])
                    elif n == EARLY_N + 4:
                        # pse0 bank partitions 0:4 are free now (staged out)
                        emit_es("e2k0")
                    elif n == K1_N - 1:
                        emit_es("m1k1")
                    elif n == K1_N:
                        emit_es("e2k1")
                    elif n == K1_N + 1:
                        emit_es("m1k2a")
                    elif n == K1_N + 2:
                        emit_es("e2k2a")
                if n == nar and rep < reps - 1:
                    for ci, (off, hp, psp) in enumerate(CH):
                        ps = psp.tile([128, CW], F32, tag="h", name=f"psr{rep}_{ci}")
                        chain(pe_prev, nc.tensor.matmul(
                            ps, wx_blk(0, 1), exg_sb[0:1, off : off + CW],
                            start=True, stop=False,
                        ))
                        ps_next[ci] = ps

            # pred for the final h
            psp_t = psPpool.tile([1, 512], F32, tag="p", name=f"psp{nar}")
            for ci, (off, _, _) in enumerate(CH):
                chain(pe_prev, nc.tensor.matmul(
                    psp_t[0:1, off : off + CW], Wd_ap, h_prev[ci], start=True, stop=True
                ))
            emit_pred_tail(nar, psp_t)
            # rows 128:220 were final after k2a: stage on the now-idle ACT
            # engine + ship on the gpsimd queue; completion overlaps the e2
            # tail below.
            nc.scalar.copy(out_stage[0:92, 512:1024], pse1)
            nc.gpsimd.dma_start(out[128:220, :], out_stage[0:92, 512:1024])

            # ------------- exponential smoothing tail -------------
            if not early_es:
                emit_es("m0k0")
                emit_es("m1k0")
                emit_es("e2k0")
                emit_es("m1k1")
                emit_es("e2k1")
                emit_es("m1k2a")
                emit_es("e2k2a")
                nc.vector.tensor_copy(out_stage[:, 0:512], pse0[0:128, :])
                nc.sync.dma_start(out[0:128, :], out_stage[:, 0:512])
                nc.vector.tensor_copy(out_stage[32:124, 512:1024], pse1)
                nc.sync.dma_start(out[128:220, :], out_stage[32:124, 512:1024])
            emit_es("e2k2b")
            # only es rows 220:223 wait on the final pred: DVE + ACT halves
            # into separate tiles (no false WAW coupling) and two small
            # closing DMAs, both on the sync queue.
            nc.vector.tensor_copy(e2a_stage[0:4, 0:256], pse2[0:4, 0:256])
            nc.scalar.copy(e2b_stage[0:4, 0:256], pse2[0:4, 256:512])
            nc.sync.dma_start(out[220:224, 0:256], e2a_stage[0:4, 0:256])
            nc.sync.dma_start(out[220:224, 256:512], e2b_stage[0:4, 0:256])

    _strip_trivial_same_engine_waits(nc)
    nc.compile()
    _PROGRAM_CACHE[key] = nc
    return nc


def _host_prep(inputs, Wx, Wh, b, Wd, bd, alpha):
    """Build the packed per-core input arrays."""
    inputs = np.ascontiguousarray(np.asarray(inputs, np.float32))
    Wx = np.asarray(Wx, np.float32)
    Wh = np.asarray(Wh, np.float32)
    b = np.asarray(b, np.float32)
    Wd = np.asarray(Wd, np.float32)
    bd_s = float(np.asarray(bd, np.float32).reshape(-1)[0])
    a = float(np.clip(np.asarray(alpha, np.float32).reshape(-1)[0], 0.0, 1.0))

    Whp = Wh + np.outer(Wd[:, 0], Wx[1, :])
    bp = b + bd_s * Wx[1, :]

    # wts: [128, 897] = Wx-block | warm block 0 | Wh | Whp | Wd
    wts_shared = np.zeros((128, 897), np.float32)
    for g in range(3):
        wts_shared[32 * g : 32 * g + 2, 0:128] = Wx
    wts_shared[:, 640:768] = Wh
    wts_shared[:, 768:896] = Whp
    wts_shared[:, 896:897] = Wd

    # L: smoothing operator [224, 224]
    La = np.zeros((NPRED, NPRED), np.float64)
    La[0, 0] = 1.0
    pw = (1.0 - a) ** np.arange(NPRED)
    for n in range(1, NPRED):
        La[n, 0] = pw[n]
        La[n, 1 : n + 1] = a * pw[:n][::-1]
    La = La.astype(np.float32)
    LaT = La.T  # [k, n]
    # lt: [128, 896], one 224-wide region per K-chunk of the ES matmul:
    #   k0  cols   0:224 -> preds k=0..127   at rows 0..127 of preds block 0
    #   k1  cols 224:448 -> preds k=128..215 at rows 0..87  of preds block 1
    #   k2a cols 448:672 -> preds k=216..219 at rows 0/32/64/96, preds23 lo
    #   k2b cols 672:896 -> preds k=220..223 at rows 0/32/64/96, preds23 hi
    direct0 = NAR - (NDIRECT - 1)  # 216
    lt = np.zeros((128, 896), np.float32)
    lt[:, 0:224] = LaT[0:128, :]
    lt[0 : direct0 - 128, 224:448] = LaT[128:direct0, :]
    for j in range(4):
        lt[32 * j, 448:672] = LaT[direct0 + j, :]
        lt[32 * j, 672:896] = LaT[direct0 + 4 + j, :]

    bias2 = np.zeros((128, 2), np.float32)
    bias2[:, 0] = b
    bias2[:, 1] = bp

    # per-core warm / exog packs
    # warm rows 2g+f = feature f of steps t with t%3==g, at column block t//3
    # (block 0 rides inside wts cols 128:640); exog rows g = exog of steps r
    # with r%3==g, at column block r//3
    x_w = inputs[:, :LAGS, :]              # [B, 32, 2]
    x_e = inputs[:, LAGS : LAGS + NAR, 0]  # [B, 223]
    in_maps = []
    shared = {"lt": lt, "bias2": bias2}
    for c in range(NCORES):
        sw = x_w[c * BL : (c + 1) * BL]  # [512, 32, 2]
        se = x_e[c * BL : (c + 1) * BL]  # [512, 223]
        wpk = np.zeros((6, WBLK * 512), np.float32)
        for t in range(LAGS):
            g, blk = t % 3, t // 3
            wpk[2 * g, blk * 512 : (blk + 1) * 512] = sw[:, t, 0]
            wpk[2 * g + 1, blk * 512 : (blk + 1) * 512] = sw[:, t, 1]
        wts_c = wts_shared.copy()
        for g in range(3):
            wts_c[32 * g : 32 * g + 2, 128:640] = wpk[2 * g : 2 * g + 2, 0:512]
        epk = np.zeros((3, EBLK * 512), np.float32)
        for r in range(NAR):
            g, blk = r % 3, r // 3
            epk[g, blk * 512 : (blk + 1) * 512] = se[:, r]
        in_maps.append({**shared, "wts": wts_c, "warm": wpk[:, 512:].copy(), "exg": epk})
    return in_maps, bd_s


def kernel(inputs, Wx, Wh, b, Wd, bd, alpha, lags, _trace=False):
    assert int(lags) == LAGS
    in_maps, bd_s = _host_prep(inputs, Wx, Wh, b, Wd, bd, alpha)
    nc = _build_program()
    import time as _time

    t0 = _time.monotonic()
    try:
        res = bass_utils.run_bass_kernel_spmd(
            nc, in_maps, core_ids=list(range(NCORES)), trace=_trace
        )
    except ModuleNotFoundError:
        # NTFF profiling hook unavailable (trimmed axon client) — run plain.
        res = bass_utils.run_bass_kernel_spmd(
            nc, in_maps, core_ids=list(range(NCORES)), trace=False
        )
    kernel.last_wall_s = _time.monotonic() - t0
    es = np.stack([r["es_out"] for r in res.results])
    # es: [8, 224, 512] -> out [4096, 224, 1]
    out = es.transpose(0, 2, 1).reshape(B, NPRED, 1) + bd_s
    if _trace:
        kernel.last_results = res
    return out.astype(np.float32)

